# revision 2
# baseline (speedup 1.0000x reference)
"""ArcFace loss kernel for Trainium2, 8-way class-sharded (partial-FC style).

Strategy
--------
- Shard the class dim (C=100000, padded to 102400) across 8 NeuronCores:
  12800 classes per core. x is replicated.
- Device (per core): L2-normalize x (sum-of-squares via a ones-matmul,
  sqrt on ACT, exact reciprocal on DVE), then a bf16 TensorEngine matmul
  x_norm^T-stationary vs W-moving, PSUM f32 accumulate over the 4 K-chunks,
  scaled on eviction by the per-class S/||W_c|| broadcast (exact f32 K=1
  matmul broadcast across partitions), written out as bf16 logits.
  Per 512-class group, the DVE Max8 instruction produces local top-8
  candidates per row (the "local topk" of the classifier shard).
- Host: gathers the logits shards, merges the per-group top-8 candidates
  into a global candidate set per row (threshold scan), recomputes the
  few candidate columns exactly in f64 to pick the final top-nc hard
  negatives and the target cosine, applies the ArcFace angular margin at
  the label position, and assembles the full [512, 100000] f32 logits.

The kernel is self-contained: shapes/sharding are hardcoded for
x[512,512] f32, W[512,100000] f32, y[512] int64, nc=10.
"""

import math

import numpy as np
import ml_dtypes

import concourse.bass as bass
import concourse.bacc as bacc
import concourse.mybir as mybir
from concourse import tile
from concourse.bass_utils import run_bass_kernel_spmd

F32 = mybir.dt.float32
F32R = mybir.dt.float32r
BF16 = mybir.dt.bfloat16
ACT = mybir.ActivationFunctionType

B = 512            # batch
D = 512            # feature dim
C = 100000         # classes
S_SCALE = 64.0
MARGIN = 0.5
EPS = 1e-12
NCORES = 8
CPAD = 102400      # C padded to 8*25*512
CLOC = CPAD // NCORES   # 12800 classes per core
G = 25             # 512-class groups per core
GW = 512           # group width
KC = 4             # K (=D) chunks of 128
BC = 4             # batch chunks of 128
NCAND = 8          # Max8 candidates per group per row

_NC_CACHE = {}


def build_nc():
    """Build the single-core SPMD graph (identical on all 8 cores)."""
    if "nc" in _NC_CACHE:
        return _NC_CACHE["nc"]

    nc = bacc.Bacc("TRN2", target_bir_lowering=False, debug=False)

    wt_ext = nc.dram_tensor("Wt", [G, 128, KC, GW], BF16, kind="ExternalInput")
    wv_ext = nc.dram_tensor("winv", [G, GW], F32, kind="ExternalInput")
    xt_ext = nc.dram_tensor("xT", [D, B], F32, kind="ExternalInput")
    lg_ext = nc.dram_tensor("logits", [B, CLOC], BF16, kind="ExternalOutput")
    cd_ext = nc.dram_tensor("cands", [B, G * NCAND], BF16, kind="ExternalOutput")

    with tile.TileContext(nc) as tc:
        with (
            tc.tile_pool(name="sbx", bufs=4) as sbx,
            tc.tile_pool(name="sbc", bufs=1) as sbc,
            tc.tile_pool(name="sbw", bufs=3) as sbw,
            tc.tile_pool(name="sbv", bufs=2) as sbv,
            tc.tile_pool(name="sbb", bufs=2) as sbb,
            tc.tile_pool(name="sbl", bufs=8) as sbl,
            tc.tile_pool(name="sbk", bufs=4) as sbk,
            tc.tile_pool(name="psA", bufs=2, space="PSUM") as psA,
            tc.tile_pool(name="psC", bufs=6, space="PSUM") as psC,
        ):
            # constants
            ones1 = sbc.tile([128, 1], F32, tag="ones1")
            nc.vector.memset(ones1[:], 1.0)
            ones128 = sbc.tile([1, 128], F32, tag="ones128")
            nc.vector.memset(ones128[:], 1.0)

            # ---- preamble: x normalization in transposed layout ----
            xt = []
            for kc in range(KC):
                t = sbx.tile([128, B], F32, tag=f"xt{kc}")
                nc.sync.dma_start(t[:], xt_ext[kc * 128:(kc + 1) * 128, :])
                xt.append(t)

            ss = psA.tile([1, B], F32, tag="psmisc")
            for kc in range(KC):
                sq = sbx.tile([128, B], F32, tag="xsq")
                nc.scalar.activation(sq[:], xt[kc][:], ACT.Square)
                nc.tensor.matmul(ss[:], ones1[:], sq[:],
                                 start=(kc == 0), stop=(kc == KC - 1))
            xnorm = sbc.tile([1, B], F32, tag="xnorm")
            nc.scalar.activation(xnorm[:], ss[:], ACT.Sqrt)
            sinv = sbc.tile([1, B], F32, tag="sinv")
            nc.vector.reciprocal(sinv[:], xnorm[:])
            bx_ps = psA.tile([128, B], F32, tag="psmisc")
            nc.tensor.matmul(bx_ps[:], ones128[:], sinv[:], start=True, stop=True)
            bsx = sbc.tile([128, B], F32, tag="bsx")
            nc.scalar.activation(bsx[:], bx_ps[:], ACT.Copy)

            xnbf = []
            for kc in range(KC):
                t = sbc.tile([128, B], BF16, tag=f"xnbf{kc}")
                nc.vector.tensor_mul(t[:], xt[kc][:], bsx[:])
                xnbf.append(t)

            # persistent per-batch-chunk candidate accumulators
            cands = []
            for bc in range(BC):
                t = sbk.tile([128, G, NCAND], BF16, tag=f"cand{bc}")
                cands.append(t)

            # ---- main loop over 512-class groups ----
            for g in range(G):
                wsb = sbw.tile([128, KC, GW], BF16, tag="wsb")
                nc.sync.dma_start(wsb[:], wt_ext[g])
                wv = sbv.tile([1, GW], F32, tag="wv")
                nc.sync.dma_start(wv[:], wv_ext[g:g + 1, :])

                # broadcast S/||W_c|| across partitions (exact f32 K=1 matmul)
                bw_ps = psA.tile([128, GW], F32, tag="psmisc")
                nc.tensor.matmul(bw_ps[:], ones128[:], wv[:], start=True, stop=True)
                bw = sbb.tile([128, GW], F32, tag="bw")
                nc.scalar.activation(bw[:], bw_ps[:], ACT.Copy)

                for bc in range(BC):
                    pc = psC.tile([128, GW], F32, tag="pscos")
                    for kc in range(KC):
                        nc.tensor.matmul(
                            pc[:],
                            xnbf[kc][:, bc * 128:(bc + 1) * 128],
                            wsb[:, kc, :],
                            start=(kc == 0), stop=(kc == KC - 1),
                        )
                    lt = sbl.tile([128, GW], BF16, tag="lt")
                    nc.vector.tensor_mul(lt[:], pc[:], bw[:])
                    nc.sync.dma_start(
                        lg_ext[bc * 128:(bc + 1) * 128, g * GW:(g + 1) * GW], lt[:]
                    )
                    nc.vector.max(cands[bc][:, g, :], lt[:])

            for bc in range(BC):
                nc.sync.dma_start(
                    cd_ext[bc * 128:(bc + 1) * 128, :],
                    cands[bc][:].rearrange("p g k -> p (g k)"),
                )

    nc.compile()
    _NC_CACHE["nc"] = nc
    return nc


def prepare_in_maps(x, W):
    """Shard + retile inputs for the 8 cores."""
    x = np.asarray(x, dtype=np.float32)
    W = np.asarray(W, dtype=np.float32)

    # per-class scale S / max(||W_c||, EPS), exact in f64
    wn = np.sqrt((W.astype(np.float64) ** 2).sum(axis=0))
    winv_full = (S_SCALE / np.maximum(wn, EPS)).astype(np.float32)
    winv_pad = np.zeros(CPAD, dtype=np.float32)
    winv_pad[:C] = winv_full

    Wb = np.zeros((D, CPAD), dtype=ml_dtypes.bfloat16)
    Wb[:, :C] = W.astype(ml_dtypes.bfloat16)

    xT = np.ascontiguousarray(x.T)

    in_maps = []
    for i in range(NCORES):
        shard = Wb[:, i * CLOC:(i + 1) * CLOC]            # [512, 12800]
        a = shard.reshape(KC, 128, G, GW)                  # [kc, d, g, m]
        Wt = np.ascontiguousarray(a.transpose(2, 1, 0, 3))  # [g, d, kc, m]
        wv = np.ascontiguousarray(
            winv_pad[i * CLOC:(i + 1) * CLOC].reshape(G, GW))
        in_maps.append({"Wt": Wt, "winv": wv, "xT": xT})
    return in_maps


def postprocess(x, W, y, nc_k, logits_sh, cands_sh):
    """Host-side gather + global topk merge + exact candidate recompute +
    ArcFace margin."""
    x64 = np.asarray(x, dtype=np.float64)
    W = np.asarray(W, dtype=np.float32)
    y = np.asarray(y).astype(np.int64)
    k = int(nc_k)

    # gather logits [B, C] f32
    full = np.concatenate(
        [np.asarray(l).astype(np.float32) for l in logits_sh], axis=1)[:, :C]

    cands = np.concatenate(
        [np.asarray(c).astype(np.float32) for c in cands_sh], axis=1)  # [B, 1600]

    # candidate threshold: rank ~48 among local-topk candidates per row
    RANK = 48
    t = -np.sort(-cands, axis=1)[:, RANK - 1]              # [B]

    xn64 = x64 / np.maximum(np.sqrt((x64 ** 2).sum(1, keepdims=True)), EPS)

    hard_idx = np.empty((B, k), dtype=np.int32)
    target = np.empty(B, dtype=np.float64)
    hits_mat = full >= t[:, None]                          # [B, C] bool
    for b in range(B):
        cols = np.nonzero(hits_mat[b])[0]
        if y[b] not in cols:
            cols = np.append(cols, y[b])
        w = W[:, cols].astype(np.float64)                  # [D, m]
        wn = np.maximum(np.sqrt((w ** 2).sum(0)), EPS)
        cos = (xn64[b] @ w) / wn                           # exact f64 cosines
        lab = cols == y[b]
        target[b] = cos[lab][0]
        neg_cols = cols[~lab]
        neg_cos = cos[~lab]
        ordv = np.argsort(-neg_cos, kind="stable")[:k]
        hard_idx[b] = neg_cols[ordv].astype(np.int32)

    # ArcFace margin at the label position (f64, then f32)
    tgt = np.clip(target, -1.0, 1.0)
    theta = np.arccos(tgt)
    mask = theta < (math.pi - MARGIN)
    margin_cos = np.where(mask, np.cos(theta + MARGIN),
                          tgt - math.sin(math.pi - MARGIN) * MARGIN)
    full[np.arange(B), y] = (S_SCALE * margin_cos).astype(np.float32)

    intra = tgt.astype(np.float32)
    return full, intra, hard_idx


def kernel(x, W, y, nc):
    bass_nc = build_nc()
    in_maps = prepare_in_maps(x, W)
    res = run_bass_kernel_spmd(bass_nc, in_maps, core_ids=list(range(NCORES)))
    logits_sh = [res.results[i]["logits"] for i in range(NCORES)]
    cands_sh = [res.results[i]["cands"] for i in range(NCORES)]
    return postprocess(x, W, y, nc, logits_sh, cands_sh)


# revision 26
# speedup vs baseline: 5.1791x; 5.1791x over previous
"""ArcFace loss kernel for Trainium2, 8-way class-sharded (partial-FC style).

Strategy
--------
- Shard the class dim (C=100000, padded to 102400) across 8 NeuronCores:
  12800 classes per core. x is replicated.
- Device (per core): L2-normalize x (sum-of-squares via a ones-matmul,
  sqrt on ACT, exact reciprocal on DVE), then a bf16 TensorEngine matmul
  x_norm^T-stationary vs W-moving, PSUM f32 accumulate over the 4 K-chunks,
  scaled on eviction by the per-class S/||W_c|| broadcast (exact f32 K=1
  matmul broadcast across partitions), written out as bf16 logits.
  Per 512-class group, the DVE Max8 instruction produces local top-8
  candidates per row (the "local topk" of the classifier shard).
- Host: gathers the logits shards, merges the per-group top-8 candidates
  into a global candidate set per row (threshold scan), recomputes the
  few candidate columns exactly in f64 to pick the final top-nc hard
  negatives and the target cosine, applies the ArcFace angular margin at
  the label position, and assembles the full [512, 100000] f32 logits.

The kernel is self-contained: shapes/sharding are hardcoded for
x[512,512] f32, W[512,100000] f32, y[512] int64, nc=10.
"""

import math

import numpy as np
import ml_dtypes

import concourse.bass as bass
import concourse.bacc as bacc
import concourse.mybir as mybir
from concourse import tile
from concourse.bass_utils import run_bass_kernel_spmd

F32 = mybir.dt.float32
F32R = mybir.dt.float32r
BF16 = mybir.dt.bfloat16
ACT = mybir.ActivationFunctionType

SBW_BUFS = 8
SBL_BUFS = 8
B = 512            # batch
D = 512            # feature dim
C = 100000         # classes
S_SCALE = 64.0
MARGIN = 0.5
EPS = 1e-12
NCORES = 8
CPAD = 102400      # C padded to 8*25*512
CLOC = CPAD // NCORES   # 12800 classes per core
G = 25             # 512-class groups per core
GW = 512           # group width
KC = 4             # K (=D) chunks of 128
BC = 4             # batch chunks of 128
NCAND = 8          # Max8 candidates per group per row

_NC_CACHE = {}


def _emit_workload(nc, pools, exts, abl=()):
    """Emit one full pass of the per-core workload."""
    sbx, sbc, sbw, sbv, sbb, sbl, sbk, psA, psC = pools
    wt_ext, wv_ext, xt_ext, lg_ext, cd_ext = exts

    # constants
    ones1 = sbc.tile([128, 1], F32, tag="ones1")
    nc.vector.memset(ones1[:], 1.0)
    ones128 = sbc.tile([1, 128], F32, tag="ones128")
    nc.vector.memset(ones128[:], 1.0)

    # ---- preamble: x normalization in transposed layout ----
    xt_all = sbx.tile([128, KC, B], F32, tag="xt")
    nc.sync.dma_start(xt_all[:], xt_ext.rearrange("(kc p) b -> p kc b", p=128))
    # per-class scale S/||W_c|| (single row; broadcast per group on GpSimd)
    wv1 = sbc.tile([1, G * GW], BF16, tag="wv1")
    nc.sync.dma_start(wv1[:], wv_ext[:])

    xsq = sbx.tile([128, KC, B], F32, tag="xsq")
    nc.scalar.activation(xsq[:], xt_all[:], ACT.Square)
    ss = psA.tile([1, B], F32, tag="psmisc")
    for kc in range(KC):
        nc.tensor.matmul(ss[:], ones1[:], xsq[:, kc, :],
                         start=(kc == 0), stop=(kc == KC - 1))
    xnorm = sbc.tile([1, B], F32, tag="xnorm")
    nc.scalar.activation(xnorm[:], ss[:], ACT.Sqrt)
    sinv = sbc.tile([1, B], F32, tag="sinv")
    nc.vector.reciprocal(sinv[:], xnorm[:])
    bx_ps = psA.tile([128, B], F32, tag="psmisc")
    nc.tensor.matmul(bx_ps[:], ones128[:], sinv[:], start=True, stop=True)
    bsx = sbc.tile([128, B], F32, tag="bsx")
    nc.scalar.activation(bsx[:], bx_ps[:], ACT.Copy)

    xn_all = sbc.tile([128, KC, B], BF16, tag="xn")
    for kc in range(KC):
        nc.vector.tensor_mul(xn_all[:, kc, :], xt_all[:, kc, :], bsx[:])

    # persistent per-batch-chunk candidate accumulators [p, bc, g, 8]
    cands = sbk.tile([128, BC, G, NCAND], BF16, tag="cands")

    # DRAM logits viewed as [p, bc, c] for one combined out-DMA per group
    lg_v = lg_ext.rearrange("(bc p) c -> p bc c", p=128)

    if "only_dma" in abl:
        for g in range(G):
            wsb = sbw.tile([128, KC, GW], BF16, tag="wsb")
            nc.sync.dma_start(wsb[:], wt_ext[g])
            nc.vector.max(cands[:, g % BC, g % G, :], wsb[:, 0, :])
        nc.vector.memset(cands[:], 0.0)
        nc.gpsimd.dma_start(
            cd_ext.rearrange("(bc p) c -> p bc c", p=128),
            cands[:].rearrange("p bc g k -> p bc (g k)"),
        )
        # consume unused tiles to satisfy Tile release checks
        nc.vector.max(cands[:, 0, 0, :], xn_all[:, 0, :])
        nc.vector.memset(cands[:, 0, 0, :], 0.0)
        nc.gpsimd.dma_start(lg_v[:, 0, 0:GW].opt(),
                            xn_all[:, 0, 0:GW])
        return

    # ---- main loop over 512-class groups ----
    for g in range(G):
        gsrc = 0 if "w_same" in abl else g
        wsb = sbw.tile([128, KC, GW], BF16, tag="wsb")
        # two dma_starts per group -> two HW queues pull concurrently
        nc.sync.dma_start(wsb[:, :2, :], wt_ext[gsrc, :, :2, :])
        nc.sync.dma_start(wsb[:, 2:, :], wt_ext[gsrc, :, 2:, :])

        bw = sbb.tile([128, GW], BF16, tag="bwg")
        nc.gpsimd.partition_broadcast(bw[:], wv1[:, g * GW:(g + 1) * GW])

        # pre-scale W tiles by S/||W_c|| (bf16 2x DVE) so PSUM holds the
        # final scaled logits and eviction is a plain ACT copy
        wsc = sbb.tile([128, KC, GW], BF16, tag="wsc")
        for kc in range(KC):
            nc.vector.tensor_mul(wsc[:, kc, :], wsb[:, kc, :], bw[:])

        for bc in range(BC):
            pc = psC.tile([128, GW], F32, tag="pscos")
            kcs = [0] if "mm1" in abl else list(range(KC))
            for j, kc in enumerate(kcs):
                nc.tensor.matmul(
                    pc[:],
                    xn_all[:, kc, bc * 128:(bc + 1) * 128],
                    wsc[:, kc, :],
                    start=(j == 0), stop=(j == len(kcs) - 1),
                )
            lt = sbl.tile([128, GW], BF16, tag="lt")
            nc.scalar.activation(lt[:], pc[:], ACT.Copy)
            if "no_max8" not in abl:
                nc.vector.max(cands[:, bc, g, :], lt[:])
            if "no_out" not in abl:
                nc.sync.dma_start(
                    lg_ext[bc * 128:(bc + 1) * 128, g * GW:(g + 1) * GW], lt[:])

    if "no_max8" in abl:
        nc.vector.memset(cands[:], 0.0)
    nc.gpsimd.dma_start(
        cd_ext.rearrange("(bc p) c -> p bc c", p=128),
        cands[:].rearrange("p bc g k -> p bc (g k)"),
    )


def build_nc(repeat=1):
    """Build the single-core SPMD graph (identical on all 8 cores).

    repeat>1 re-emits the whole workload that many times in one NEFF —
    used only for timing (amortizes the per-dispatch overhead)."""
    if repeat in _NC_CACHE:
        return _NC_CACHE[repeat]

    nc = bacc.Bacc("TRN2", target_bir_lowering=False, debug=False)

    wt_ext = nc.dram_tensor("Wt", [G, 128, KC, GW], BF16, kind="ExternalInput")
    wv_ext = nc.dram_tensor("winv", [1, G * GW], BF16, kind="ExternalInput")
    xt_ext = nc.dram_tensor("xT", [D, B], F32, kind="ExternalInput")
    lg_ext = nc.dram_tensor("logits", [B, CLOC], BF16, kind="ExternalOutput")
    cd_ext = nc.dram_tensor("cands", [B, G * NCAND], BF16, kind="ExternalOutput")
    exts = (wt_ext, wv_ext, xt_ext, lg_ext, cd_ext)

    with tile.TileContext(nc) as tc:
        with (
            tc.tile_pool(name="sbx", bufs=2) as sbx,
            tc.tile_pool(name="sbc", bufs=1) as sbc,
            tc.tile_pool(name="sbw", bufs=SBW_BUFS) as sbw,
            tc.tile_pool(name="sbv", bufs=2) as sbv,
            tc.tile_pool(name="sbb", bufs=4) as sbb,
            tc.tile_pool(name="sbl", bufs=SBL_BUFS) as sbl,
            tc.tile_pool(name="sbk", bufs=1) as sbk,
            tc.tile_pool(name="psA", bufs=1, space="PSUM") as psA,
            tc.tile_pool(name="psC", bufs=7, space="PSUM") as psC,
        ):
            pools = (sbx, sbc, sbw, sbv, sbb, sbl, sbk, psA, psC)
            if isinstance(repeat, tuple):       # ("loop", R, abl): device loop
                r = repeat[1]
                abl = repeat[2] if len(repeat) > 2 else ()
                with tc.For_i(0, r, 1):
                    _emit_workload(nc, pools, exts, abl)
            else:
                for _ in range(repeat):
                    _emit_workload(nc, pools, exts)

    nc.compile()
    _NC_CACHE[repeat] = nc
    return nc


def prepare_in_maps(x, W):
    """Shard + retile inputs for the 8 cores."""
    x = np.asarray(x, dtype=np.float32)
    W = np.asarray(W, dtype=np.float32)

    # per-class scale S / max(||W_c||, EPS), exact in f64
    wn = np.sqrt((W.astype(np.float64) ** 2).sum(axis=0))
    winv_full = (S_SCALE / np.maximum(wn, EPS)).astype(np.float32)
    winv_pad = np.zeros(CPAD, dtype=np.float32)
    winv_pad[:C] = winv_full

    Wb = np.zeros((D, CPAD), dtype=ml_dtypes.bfloat16)
    Wb[:, :C] = W.astype(ml_dtypes.bfloat16)

    xT = np.ascontiguousarray(x.T)

    in_maps = []
    for i in range(NCORES):
        shard = Wb[:, i * CLOC:(i + 1) * CLOC]            # [512, 12800]
        a = shard.reshape(KC, 128, G, GW)                  # [kc, d, g, m]
        Wt = np.ascontiguousarray(a.transpose(2, 1, 0, 3))  # [g, d, kc, m]
        wv = np.ascontiguousarray(
            winv_pad[i * CLOC:(i + 1) * CLOC].astype(ml_dtypes.bfloat16)[None, :])
        in_maps.append({"Wt": Wt, "winv": wv, "xT": xT})
    return in_maps


def postprocess(x, W, y, nc_k, logits_sh, cands_sh):
    """Host-side gather + global topk merge + exact candidate recompute +
    ArcFace margin."""
    x64 = np.asarray(x, dtype=np.float64)
    W = np.asarray(W, dtype=np.float32)
    y = np.asarray(y).astype(np.int64)
    k = int(nc_k)

    # gather logits [B, C] f32
    full = np.concatenate(
        [np.asarray(l).astype(np.float32) for l in logits_sh], axis=1)[:, :C]

    cands = np.concatenate(
        [np.asarray(c).astype(np.float32) for c in cands_sh], axis=1)  # [B, 1600]

    # candidate threshold: rank ~48 among local-topk candidates per row
    RANK = 48
    t = -np.sort(-cands, axis=1)[:, RANK - 1]              # [B]

    xn64 = x64 / np.maximum(np.sqrt((x64 ** 2).sum(1, keepdims=True)), EPS)

    hard_idx = np.empty((B, k), dtype=np.int32)
    target = np.empty(B, dtype=np.float64)
    hits_mat = full >= t[:, None]                          # [B, C] bool
    for b in range(B):
        cols = np.nonzero(hits_mat[b])[0]
        if y[b] not in cols:
            cols = np.append(cols, y[b])
        w = W[:, cols].astype(np.float64)                  # [D, m]
        wn = np.maximum(np.sqrt((w ** 2).sum(0)), EPS)
        cos = (xn64[b] @ w) / wn                           # exact f64 cosines
        lab = cols == y[b]
        target[b] = cos[lab][0]
        neg_cols = cols[~lab]
        neg_cos = cos[~lab]
        ordv = np.argsort(-neg_cos, kind="stable")[:k]
        hard_idx[b] = neg_cols[ordv].astype(np.int32)

    # ArcFace margin at the label position (f64, then f32)
    tgt = np.clip(target, -1.0, 1.0)
    theta = np.arccos(tgt)
    mask = theta < (math.pi - MARGIN)
    margin_cos = np.where(mask, np.cos(theta + MARGIN),
                          tgt - math.sin(math.pi - MARGIN) * MARGIN)
    full[np.arange(B), y] = (S_SCALE * margin_cos).astype(np.float32)

    intra = tgt.astype(np.float32)
    return full, intra, hard_idx


def kernel(x, W, y, nc):
    bass_nc = build_nc()
    in_maps = prepare_in_maps(x, W)
    res = run_bass_kernel_spmd(bass_nc, in_maps, core_ids=list(range(NCORES)))
    logits_sh = [res.results[i]["logits"] for i in range(NCORES)]
    cands_sh = [res.results[i]["cands"] for i in range(NCORES)]
    return postprocess(x, W, y, nc, logits_sh, cands_sh)
